# revision 7
# baseline (speedup 1.0000x reference)
"""Trainium2 Bass kernel for an MoE routing module.

Strategy: data-parallel over the batch — each of the 8 NeuronCores runs the
full pipeline (gating -> top-2 -> expert MLPs) for its 8 samples. All
data-dependent expert selection is done with indirect-DMA gathers driven by
index tiles computed on device; there are no collectives and no registers.

Host-side prep is limited to dtype casts and weight re-layouts so that one
[128,1] index tile (value e*128+p) gathers a whole expert's weights with 128
fat descriptors:
    exp_w1 [E,D,H]  -> [E*128, 8*1024]  row e*128+p = W1[e, t*128+p, :], t-major
    exp_w2 [E,H,C]  -> [E*128, 8*16]
    exp_b1 [E,H]    -> [E*128, 8]
Expert tables/weights are bf16 (fp32 PSUM accumulation); the gating path is
kept fp32 so top-2 selection matches the fp32 reference.

HW gotcha (verified on device): indirect DMA consumes exactly ONE index per
destination partition — multi-index-per-partition gathers return garbage, so
every gather here uses a [P,1] index slice.
"""

import os
import sys

for _p in ("/opt/trn_rl_repo", "/root/.axon_site/_ro/trn_rl_repo"):
    if os.path.isdir(_p) and _p not in sys.path:
        sys.path.insert(0, _p)

import numpy as np

import concourse.bacc as bacc
import concourse.tile as tile
import concourse.mybir as mybir
from concourse.bass import IndirectOffsetOnAxis
from concourse.bass_utils import run_bass_kernel_spmd
from concourse.masks import make_identity

F32 = mybir.dt.float32
BF16 = mybir.dt.bfloat16
I32 = mybir.dt.int32
U32 = mybir.dt.uint32

V, D, H, E, C, TOPK = 16000, 1024, 1024, 8, 16, 2
B, S = 64, 512
GATE_H = 256
NCORES = 8
BL = B // NCORES          # samples per core
DT = D // 128             # 8 d-tiles
HT = H // 128             # 8 h-tiles
ST = S // 128             # 4 s-tiles
MT = GATE_H // 128        # 2 gate-hidden tiles
NPAIR = BL * TOPK         # 16 (sample, k) pairs per core

_compiled = {}
last_results = None       # BassKernelResults of the most recent run (for test.py)


def build_program(reps=1):
    """reps>1 repeats the whole compute body (benchmarking aid: the axon
    dispatch overhead is ~100x the kernel, so per-iteration time is measured
    as (t(reps=K) - t(reps=1)) / (K-1))."""
    nc = bacc.Bacc("TRN2", target_bir_lowering=False, debug=False, num_devices=NCORES)
    act = mybir.ActivationFunctionType

    x_t = nc.dram_tensor("x_loc", [BL, S], I32, kind="ExternalInput")
    emb_t = nc.dram_tensor("emb", [V, D], F32, kind="ExternalInput")
    eemb_t = nc.dram_tensor("eemb", [E * V, D], BF16, kind="ExternalInput")
    ew1_t = nc.dram_tensor("ew1", [E * 128, DT * H], BF16, kind="ExternalInput")
    ew2_t = nc.dram_tensor("ew2", [E * 128, HT * C], F32, kind="ExternalInput")
    b1_t = nc.dram_tensor("b1r", [E * 128, HT], F32, kind="ExternalInput")
    b2_t = nc.dram_tensor("b2r", [E * C, 1], F32, kind="ExternalInput")
    gw1_t = nc.dram_tensor("gw1", [D, GATE_H], F32, kind="ExternalInput")
    gb1_t = nc.dram_tensor("gb1", [128, MT], F32, kind="ExternalInput")
    gw2_t = nc.dram_tensor("gw2", [GATE_H, E], F32, kind="ExternalInput")
    gb2_t = nc.dram_tensor("gb2", [E, 1], F32, kind="ExternalInput")
    out_t = nc.dram_tensor("out", [BL, C], F32, kind="ExternalOutput")

    with tile.TileContext(nc) as tc:
        with (
            tc.tile_pool(name="const", bufs=1) as cpool,
            tc.tile_pool(name="dram", bufs=1, space="DRAM") as dpool,
        ):
            # ---- constants ----
            id_bf = cpool.tile([128, 128], BF16)
            make_identity(nc, id_bf[:, :])
            id_f = cpool.tile([128, 128], F32)
            make_identity(nc, id_f[:, :])
            ones_k = cpool.tile([128, 1], F32)      # lhsT for partition-sum MMs
            nc.vector.memset(ones_k[:, :], 1.0)
            ones_m = cpool.tile([1, 128], F32)      # lhsT for K=1 broadcast MMs
            nc.vector.memset(ones_m[:, :], 1.0)
            iota_p = cpool.tile([128, 1], I32)      # value = partition index
            nc.gpsimd.iota(iota_p[:, :], pattern=[[0, 1]], base=0, channel_multiplier=1)

            # token ids, transposed: xt[p, b, t] = x[b, t*128+p]
            xt = cpool.tile([128, BL, ST], I32)
            nc.sync.dma_start(
                out=xt[:, :, :], in_=x_t[:, :].rearrange("b (t p) -> p b t", p=128)
            )

            gb1_sb = cpool.tile([128, MT], F32)
            nc.sync.dma_start(out=gb1_sb[:, :], in_=gb1_t[:, :])
            gb2_sb = cpool.tile([E, 1], F32)
            nc.sync.dma_start(out=gb2_sb[:, :], in_=gb2_t[:, :])
            gw1_sb = cpool.tile([128, DT, GATE_H], F32)
            nc.sync.dma_start(
                out=gw1_sb[:, :, :], in_=gw1_t[:, :].rearrange("(j p) g -> p j g", p=128)
            )
            gw2_sb = cpool.tile([128, MT, E], F32)
            nc.sync.dma_start(
                out=gw2_sb[:, :, :], in_=gw2_t[:, :].rearrange("(m p) e -> p m e", p=128)
            )

            for rep in range(reps):
                _body_once(
                    nc, tc, act, rep,
                    cpool=cpool, dpool=dpool,
                    id_bf=id_bf, id_f=id_f, ones_k=ones_k, ones_m=ones_m,
                    iota_p=iota_p, xt=xt, gb1_sb=gb1_sb, gb2_sb=gb2_sb,
                    gw1_sb=gw1_sb, gw2_sb=gw2_sb,
                    emb_t=emb_t, eemb_t=eemb_t, ew1_t=ew1_t, ew2_t=ew2_t,
                    b1_t=b1_t, b2_t=b2_t, out_t=out_t,
                )

    nc.compile()
    return nc


def _body_once(nc, tc, act, rep, *, cpool, dpool, id_bf, id_f, ones_k, ones_m,
               iota_p, xt, gb1_sb, gb2_sb, gw1_sb, gw2_sb,
               emb_t, eemb_t, ew1_t, ew2_t, b1_t, b2_t, out_t):
    sfx = f"_r{rep}"
    with tc.tile_pool(name=f"persist{sfx}", bufs=1) as ppool:
        # broadcast tiles (filled after gating)
        BCi = ppool.tile([128, BL * 8], I32)
        BCf = ppool.tile([128, BL * 8], F32)
        out_acc = ppool.tile([C, BL], F32)
        nc.vector.memset(out_acc[:, :], 0.0)

        # ================= gating (fp32) =================
        with (
            tc.tile_pool(name=f"gat{sfx}", bufs=2) as gpool,
            tc.tile_pool(name=f"gat1{sfx}", bufs=2) as g1pool,
            tc.tile_pool(name=f"gsb{sfx}", bufs=1) as gspool,
            tc.tile_pool(name=f"gps{sfx}", bufs=2, space="PSUM") as gps,
            tc.tile_pool(name=f"gpss{sfx}", bufs=1, space="PSUM") as gps_s,
        ):
            pooled = gspool.tile([1, BL * D], F32)
            for b in range(BL):
                gtok = gpool.tile([128, ST, D], F32, tag="gtok")
                # one index per partition per indirect DMA (multi-index
                # per partition diverges on HW)
                for t in range(ST):
                    nc.gpsimd.indirect_dma_start(
                        out=gtok[:, t, :],
                        out_offset=None,
                        in_=emb_t[:, :],
                        in_offset=IndirectOffsetOnAxis(
                            ap=xt[:, b, t : t + 1], axis=0
                        ),
                    )
                t01 = g1pool.tile([128, D], F32, tag="t01")
                t23 = g1pool.tile([128, D], F32, tag="t23")
                nc.vector.tensor_add(t01[:, :], gtok[:, 0, :], gtok[:, 1, :])
                nc.vector.tensor_add(t23[:, :], gtok[:, 2, :], gtok[:, 3, :])
                cb = g1pool.tile([128, D], F32, tag="cb")
                nc.vector.tensor_add(cb[:, :], t01[:, :], t23[:, :])
                for h in range(2):
                    pp = gps.tile([1, 512], F32, tag="pool_ps")
                    nc.tensor.matmul(
                        out=pp[:, :],
                        lhsT=ones_k[:, :],
                        rhs=cb[:, h * 512 : (h + 1) * 512],
                        start=True,
                        stop=True,
                    )
                    # mean over S
                    nc.scalar.activation(
                        out=pooled[0:1, b * D + h * 512 : b * D + (h + 1) * 512],
                        in_=pp[:, :],
                        func=act.Copy,
                        scale=1.0 / S,
                    )

            # pooled^T [d, b] via K=1 matmuls into one psum tile
            pt_ps = gps_s.tile([128, DT * BL], F32, tag="pt")
            for b in range(BL):
                for j in range(DT):
                    nc.tensor.matmul(
                        out=pt_ps[:, j * BL + b : j * BL + b + 1],
                        lhsT=pooled[0:1, b * D + j * 128 : b * D + (j + 1) * 128],
                        rhs=ones_m[0:1, 0:1],
                        start=True,
                        stop=True,
                    )
            pts = gspool.tile([128, DT * BL], F32)
            nc.vector.tensor_copy(pts[:, :], pt_ps[:, :])

            # gate layer 1 + relu
            hR = gspool.tile([128, MT, BL], F32)
            for m in range(MT):
                h_ps = gps_s.tile([128, BL], F32, tag="h1")
                for j in range(DT):
                    nc.tensor.matmul(
                        out=h_ps[:, :],
                        lhsT=gw1_sb[:, j, m * 128 : (m + 1) * 128],
                        rhs=pts[:, j * BL : (j + 1) * BL],
                        start=(j == 0),
                        stop=(j == DT - 1),
                    )
                nc.scalar.activation(
                    out=hR[:, m, :],
                    in_=h_ps[:, :],
                    func=act.Relu,
                    bias=gb1_sb[:, m : m + 1],
                )

            # gate layer 2 -> logits [e, b]
            l_ps = gps_s.tile([E, BL], F32, tag="l2")
            for m in range(MT):
                nc.tensor.matmul(
                    out=l_ps[:, :],
                    lhsT=gw2_sb[:, m, :],
                    rhs=hR[:, m, :],
                    start=(m == 0),
                    stop=(m == MT - 1),
                )
            l_sb = gspool.tile([E, BL], F32)
            nc.scalar.activation(
                out=l_sb[:, :], in_=l_ps[:, :], func=act.Identity,
                bias=gb2_sb[:, 0:1],
            )
            # transpose logits -> [b, e]
            lt_ps = gps_s.tile([BL, E], F32, tag="lt")
            nc.tensor.matmul(
                out=lt_ps[:, :], lhsT=l_sb[:, :], rhs=id_f[0:E, 0:E],
                start=True, stop=True,
            )
            lt_sb = gspool.tile([BL, E], F32)
            nc.vector.tensor_copy(lt_sb[:, :], lt_ps[:, :])

            # top-2 of logits == top-2 of softmax (monotone)
            mx = gspool.tile([BL, 8], F32)
            mi = gspool.tile([BL, 8], U32)
            nc.vector.max_with_indices(mx[:, :], mi[:, :], lt_sb[:, :])

            # renormalized top-2 softmax weights:
            # rw1 = 1/(1+exp(l2-l1)), rw2 = exp(l2-l1)/(1+exp(l2-l1))
            dlt = gspool.tile([BL, 1], F32)
            nc.vector.tensor_sub(dlt[:, :], mx[:, 1:2], mx[:, 0:1])
            q = gspool.tile([BL, 1], F32)
            nc.scalar.activation(out=q[:, :], in_=dlt[:, :], func=act.Exp)
            sden = gspool.tile([BL, 1], F32)
            nc.vector.tensor_scalar_add(sden[:, :], q[:, :], 1.0)
            rw1 = gspool.tile([BL, 1], F32)
            nc.vector.reciprocal(rw1[:, :], sden[:, :])
            rw2 = gspool.tile([BL, 1], F32)
            nc.vector.tensor_mul(rw2[:, :], q[:, :], rw1[:, :])

            # pack per-(b,k) scalars: cols b*8 + {0,1}=e*V, {2,3}=e*128,
            # {4,5}=e*C, {6,7}=rw
            ei_f = gspool.tile([BL, TOPK], F32)
            nc.vector.tensor_copy(ei_f[:, :], mi[:, 0:TOPK])
            vals = gspool.tile([BL, 8], F32)
            nc.vector.tensor_scalar_mul(vals[:, 0:2], ei_f[:, :], float(V))
            nc.vector.tensor_scalar_mul(vals[:, 2:4], ei_f[:, :], 128.0)
            nc.vector.tensor_scalar_mul(vals[:, 4:6], ei_f[:, :], float(C))
            nc.vector.tensor_copy(vals[:, 6:7], rw1[:, :])
            nc.vector.tensor_copy(vals[:, 7:8], rw2[:, :])

            # broadcast across partitions: bounce through DRAM to get a
            # flat [1, 64] row, then K=1 matmul against ones.
            scratch = dpool.tile([BL, 8], F32, tag=f"scratch{sfx}")
            nc.sync.dma_start(out=scratch[:, :], in_=vals[:, :])
            flat = gspool.tile([1, BL * 8], F32)
            nc.sync.dma_start(
                out=flat[0:1, :].rearrange("p (b c) -> p b c", b=BL),
                in_=scratch[:, :],
            )
            bc_ps = gps_s.tile([128, BL * 8], F32, tag="bc")
            nc.tensor.matmul(
                out=bc_ps[:, :], lhsT=ones_m[:, :], rhs=flat[0:1, :],
                start=True, stop=True,
            )
            nc.vector.tensor_copy(BCf[:, :], bc_ps[:, :])
            nc.vector.tensor_copy(BCi[:, :], bc_ps[:, :])  # cast f32->i32

        # ================= experts (bf16) =================
        with (
            tc.tile_pool(name=f"exi{sfx}", bufs=3) as xipool,
            tc.tile_pool(name=f"etok{sfx}", bufs=2) as tokpool,
            tc.tile_pool(name=f"ew{sfx}", bufs=2) as wpool,
            tc.tile_pool(name=f"ett{sfx}", bufs=2) as ttpool,
            tc.tile_pool(name=f"esm{sfx}", bufs=3) as smpool,
            tc.tile_pool(name=f"ejunk{sfx}", bufs=2) as junkpool,
            tc.tile_pool(name=f"epst{sfx}", bufs=2, space="PSUM") as eps_t,
            tc.tile_pool(name=f"epsz{sfx}", bufs=2, space="PSUM") as eps_z,
            tc.tile_pool(name=f"epso{sfx}", bufs=2, space="PSUM") as eps_o,
        ):
            for b in range(BL):
                for k in range(TOPK):
                    cEV = b * 8 + k
                    cE128 = b * 8 + 2 + k
                    cEC = b * 8 + 4 + k
                    cRW = b * 8 + 6 + k

                    tok_idx = xipool.tile([128, ST], I32, tag="tok_idx")
                    nc.vector.tensor_add(
                        tok_idx[:, :],
                        xt[:, b, :],
                        BCi[:, cEV : cEV + 1].to_broadcast([128, ST]),
                    )
                    w_idx = xipool.tile([128, 1], I32, tag="w_idx")
                    nc.vector.tensor_add(
                        w_idx[:, :], iota_p[:, :], BCi[:, cE128 : cE128 + 1]
                    )
                    b2_idx = xipool.tile([C, 1], I32, tag="b2_idx")
                    nc.vector.tensor_add(
                        b2_idx[:, :], iota_p[0:C, :], BCi[0:C, cEC : cEC + 1]
                    )

                    tok = tokpool.tile([128, ST, D], BF16, tag="tok")
                    for t in range(ST):
                        nc.gpsimd.indirect_dma_start(
                            out=tok[:, t, :],
                            out_offset=None,
                            in_=eemb_t[:, :],
                            in_offset=IndirectOffsetOnAxis(
                                ap=tok_idx[:, t : t + 1], axis=0
                            ),
                        )
                    w1g = wpool.tile([128, DT * H], BF16, tag="w1g")
                    nc.gpsimd.indirect_dma_start(
                        out=w1g[:, :],
                        out_offset=None,
                        in_=ew1_t[:, :],
                        in_offset=IndirectOffsetOnAxis(ap=w_idx[:, :], axis=0),
                    )
                    w2g = smpool.tile([128, HT * C], F32, tag="w2g")
                    nc.gpsimd.indirect_dma_start(
                        out=w2g[:, :],
                        out_offset=None,
                        in_=ew2_t[:, :],
                        in_offset=IndirectOffsetOnAxis(ap=w_idx[:, :], axis=0),
                    )
                    b1g = smpool.tile([128, HT], F32, tag="b1g")
                    nc.gpsimd.indirect_dma_start(
                        out=b1g[:, :],
                        out_offset=None,
                        in_=b1_t[:, :],
                        in_offset=IndirectOffsetOnAxis(ap=w_idx[:, :], axis=0),
                    )
                    b2g = smpool.tile([C, 1], F32, tag="b2g")
                    nc.gpsimd.indirect_dma_start(
                        out=b2g[:, :],
                        out_offset=None,
                        in_=b2_t[:, :],
                        in_offset=IndirectOffsetOnAxis(ap=b2_idx[:, :], axis=0),
                    )

                    # transpose tok -> tokT[d, s] via matmul against identity
                    tokT = ttpool.tile([128, DT, S], BF16, tag="tokT")
                    for j in range(DT):
                        tp = eps_t.tile([128, S], F32, tag="tp")
                        for t in range(ST):
                            nc.tensor.matmul(
                                out=tp[:, t * 128 : (t + 1) * 128],
                                lhsT=tok[:, t, j * 128 : (j + 1) * 128],
                                rhs=id_bf[:, :],
                                start=True,
                                stop=True,
                            )
                        nc.vector.tensor_copy(tokT[:, j, :], tp[:, :])

                    # z[h_tile] = relu(tokT.T @ W1 + b1); accumulate sum over s
                    pacc = smpool.tile([128, HT], F32, tag="pacc")
                    for j2 in range(HT):
                        z_ps = eps_z.tile([128, S], F32, tag="z")
                        for t in range(DT):
                            nc.tensor.matmul(
                                out=z_ps[:, :],
                                lhsT=w1g[:, t * H + j2 * 128 : t * H + (j2 + 1) * 128],
                                rhs=tokT[:, t, :],
                                start=(t == 0),
                                stop=(t == DT - 1),
                            )
                        zjunk = junkpool.tile([128, S], BF16, tag="zjunk")
                        nc.scalar.activation(
                            out=zjunk[:, :],
                            in_=z_ps[:, :],
                            func=act.Relu,
                            bias=b1g[:, j2 : j2 + 1],
                            accum_out=pacc[:, j2 : j2 + 1],
                        )

                    psc = smpool.tile([128, HT], F32, tag="psc")
                    nc.vector.tensor_scalar_mul(psc[:, :], pacc[:, :], 1.0 / S)

                    eo_ps = eps_o.tile([C, 1], F32, tag="eo")
                    for j2 in range(HT):
                        nc.tensor.matmul(
                            out=eo_ps[:, :],
                            lhsT=w2g[:, j2 * C : (j2 + 1) * C],
                            rhs=psc[:, j2 : j2 + 1],
                            start=(j2 == 0),
                            stop=(j2 == HT - 1),
                        )
                    eo1 = smpool.tile([C, 1], F32, tag="eo1")
                    nc.scalar.activation(
                        out=eo1[:, :], in_=eo_ps[:, :], func=act.Identity,
                        bias=b2g[:, 0:1],
                    )
                    eo2 = smpool.tile([C, 1], F32, tag="eo2")
                    nc.vector.tensor_mul(eo2[:, :], eo1[:, :], BCf[0:C, cRW : cRW + 1])
                    nc.vector.tensor_add(
                        out_acc[:, b : b + 1], out_acc[:, b : b + 1], eo2[:, :]
                    )

        nc.sync.dma_start(
            out=out_t[:, :].rearrange("b c -> c b"), in_=out_acc[:, :]
        )


def _prep_inputs(inputs):
    """Host-side dtype casts + re-layouts shared by all cores."""
    import ml_dtypes

    f32 = np.float32
    bf16 = ml_dtypes.bfloat16

    x = np.asarray(inputs["x"]).astype(np.int32)
    emb = np.asarray(inputs["emb"], dtype=f32)
    exp_emb = np.ascontiguousarray(
        np.asarray(inputs["exp_emb"], dtype=f32).reshape(E * V, D)
    ).astype(bf16)
    w1 = np.asarray(inputs["exp_w1"], dtype=f32)          # [E, D, H]
    ew1 = np.ascontiguousarray(
        w1.reshape(E, DT, 128, H).transpose(0, 2, 1, 3).reshape(E * 128, DT * H)
    ).astype(bf16)
    w2 = np.asarray(inputs["exp_w2"], dtype=f32)          # [E, H, C]
    ew2 = np.ascontiguousarray(
        w2.reshape(E, HT, 128, C).transpose(0, 2, 1, 3).reshape(E * 128, HT * C)
    )
    b1 = np.asarray(inputs["exp_b1"], dtype=f32)          # [E, H]
    b1r = np.ascontiguousarray(
        b1.reshape(E, HT, 128).transpose(0, 2, 1).reshape(E * 128, HT)
    )
    b2r = np.ascontiguousarray(np.asarray(inputs["exp_b2"], dtype=f32).reshape(E * C, 1))
    gw1 = np.ascontiguousarray(np.asarray(inputs["gate_w1"], dtype=f32))
    gb1 = np.ascontiguousarray(
        np.asarray(inputs["gate_b1"], dtype=f32).reshape(MT, 128).T
    )
    gw2 = np.ascontiguousarray(np.asarray(inputs["gate_w2"], dtype=f32))
    gb2 = np.ascontiguousarray(np.asarray(inputs["gate_b2"], dtype=f32).reshape(E, 1))

    shared = dict(
        emb=emb, eemb=exp_emb, ew1=ew1, ew2=ew2, b1r=b1r, b2r=b2r,
        gw1=gw1, gb1=gb1, gw2=gw2, gb2=gb2,
    )
    return x, shared


def kernel(**inputs) -> np.ndarray:
    global last_results
    if "nc" not in _compiled:
        _compiled["nc"] = build_program()
    nc = _compiled["nc"]

    x, shared = _prep_inputs(inputs)
    in_maps = [
        {"x_loc": np.ascontiguousarray(x[c * BL : (c + 1) * BL]), **shared}
        for c in range(NCORES)
    ]
    res = run_bass_kernel_spmd(nc, in_maps, list(range(NCORES)))
    last_results = res
    out = np.concatenate([res.results[c]["out"] for c in range(NCORES)], axis=0)
    return np.ascontiguousarray(out.astype(np.float32))
